# revision 2
# baseline (speedup 1.0000x reference)
"""Stack-style neural memory kernel for Trainium2 (8 NeuronCores, SPMD).

Reference semantics: at step t, push (d1,v1),(d2,v2); read up to total
strength u_t from the top of the stack; pop strength u_t.  The read
summary is linear in the pushed values:

    out[t,b,:] = sum_j W[t,j,b] * V[j,b,:]      (j = slot index, 2T slots)

where the weights W depend only on the (T,B,1)-sized strength tensors
(u,d1,d2).  W is computed on host (tiny sequential bookkeeping, ~4M
scalar ops; it also needs a global max over the whole batch, which would
otherwise force cross-core communication).  The device does the
memory-heavy part: per batch element a (T x 2T) @ (2T x R) matmul,
batch-parallel across 8 cores with no communication.

All device traffic is bf16 (W, V in; out back out, upcast on host):
per-core 5.25MB in + 2.1MB out = 7.3MB -> ~20.5us at the ~358GB/s HBM
limit, vs 14MB/~40us for the fp32 variant.  bf16 also puts the PE at
1 cycle/row instead of fp32's 4 (PE ~6.8us, fully hidden under DMA).
Quantization error ~0.4% rms, well under the 2e-2 gate.
"""

import numpy as np
import ml_dtypes

BF16 = ml_dtypes.bfloat16

T, B, R = 128, 128, 512
NSLOTS = 2 * T
N_CORES = 8
BSH = B // N_CORES  # batch shard per core
GRP = 4             # batches per DMA group
NGRP = BSH // GRP
WM = 256            # per-batch lhsT columns: chunk0 + chunk1
BW = WM + 2 * 512   # per-batch fused row: lhsT pack + both v chunks

_NC_CACHE = {}


def _compute_weights(u, d1, d2):
    """W[t, j, b]: read weight of slot j at step t (float32 (T, 2T, B))."""
    uu = u[:, :, 0]
    S = np.zeros((NSLOTS, B), np.float32)
    W = np.empty((T, NSLOTS, B), np.float32)
    for t in range(T):
        S[2 * t] = d1[t, :, 0]
        S[2 * t + 1] = d2[t, :, 0]
        # strength of slots above j (stack top = highest index first)
        c = np.cumsum(S[::-1], axis=0)[::-1]
        cum = c - S
        avail = uu[t][None, :] - cum
        # reference takes a GLOBAL max over the batch for the read scale
        scal = avail.max(axis=1)
        Wt = np.minimum(S, scal[:, None])
        Wt[2 * t + 2:] = 0.0  # slots not yet pushed hold V=0 in the reference
        W[t] = Wt
        # pop u_t: elementwise depletion, same slot order, same cum
        S -= np.minimum(S, np.maximum(0.0, avail))
    return W


def _build_nc(reps=1, loop_n=1):
    import contextlib

    from concourse import bacc, tile, mybir

    DT = mybir.dt.bfloat16
    nc = bacc.Bacc(None)
    # One fused, fully partition-contiguous load stream per group:
    # wv[g, k, bi*BW + m]: m<128 chunk0 lhsT, 128<=m<256 chunk1 lhsT,
    # 256<=m<768 v chunk0 row, 768<=m<1280 v chunk1 row.  Per-partition
    # contiguous run = GRP*BW*2 bytes (10KB for GRP=4) -> large DMA
    # descriptors on both DRAM and SBUF sides.
    wv = nc.declare_dram_parameter("wv", [NGRP, 128, GRP * BW], DT, isOutput=False)
    # output t-major per group: o[g, t, bi*512 + r] (4KB runs per partition)
    o = nc.declare_dram_parameter("o", [NGRP, 128, GRP * 512], DT, isOutput=True)

    with tile.TileContext(nc) as tc:
        with (
            tc.tile_pool(name="wvp", bufs=3) as wvp,
            tc.tile_pool(name="op", bufs=3) as op,
            tc.tile_pool(name="ps", bufs=8, space="PSUM") as ps,
        ):
            loop_cm = (
                tc.For_i(0, loop_n, 1) if loop_n > 1 else contextlib.nullcontext()
            )
            with loop_cm:
                for rep in range(reps):
                    for g in range(NGRP):
                        # alternate the two HWDGE rings (SP / Act) by group
                        # parity so loads and stores stream on both rings
                        ld = nc.sync if g % 2 == 0 else nc.scalar
                        ld2 = nc.scalar if g % 2 == 0 else nc.sync
                        st = nc.scalar if g % 2 == 0 else nc.sync
                        wv_t = wvp.tile([128, GRP, BW], DT, tag="wv")
                        # split each group load across BOTH HWDGE rings:
                        # halves the wait before the group's first matmul
                        wv_g = wv[g].rearrange("k (b m) -> k b m", m=BW)
                        ld.dma_start(wv_t[:, 0 : GRP // 2], wv_g[:, 0 : GRP // 2])
                        ld2.dma_start(wv_t[:, GRP // 2 :], wv_g[:, GRP // 2 :])
                        out_t = op.tile([128, GRP, 512], DT, tag="out")
                        for bi in range(GRP):
                            acc = ps.tile([128, 512], mybir.dt.float32)
                            nc.tensor.matmul(
                                acc[:],
                                wv_t[:, bi, 0:128],
                                wv_t[:, bi, 256:768],
                                start=True,
                                stop=False,
                            )
                            nc.tensor.matmul(
                                acc[:],
                                wv_t[:, bi, 128:256],
                                wv_t[:, bi, 768:1280],
                                start=False,
                                stop=True,
                            )
                            nc.vector.tensor_copy(out_t[:, bi], acc[:])
                        st.dma_start(o[g], out_t[:])
    nc.compile()
    return nc


def _make_in_maps(u, d1, d2, v1, v2):
    W = _compute_weights(u, d1, d2)  # (T, 2T, B)

    Vfull = np.empty((NSLOTS, B, R), np.float32)
    Vfull[0::2] = v1
    Vfull[1::2] = v2

    in_maps = []
    for c in range(N_CORES):
        gb = slice(c * BSH, (c + 1) * BSH)
        # fused per-batch row [k, m]: 0:128 = W[m, k, b] (chunk0 lhsT),
        # 128:256 = W[m, 128+k, b] (chunk1 lhsT), 256:768 = Vfull[k, b, r],
        # 768:1280 = Vfull[128+k, b, r].
        Wc = W[:, :, gb]          # (T, 256, BSH)
        Vc = Vfull[:, gb, :]      # (256, BSH, R)
        pack = np.empty((BSH, 128, BW), np.float32)
        pack[:, :, 0:128] = Wc[:, 0:128, :].transpose(2, 1, 0)
        pack[:, :, 128:256] = Wc[:, 128:256, :].transpose(2, 1, 0)
        pack[:, :, 256:768] = Vc[0:128].transpose(1, 0, 2)
        pack[:, :, 768:1280] = Vc[128:256].transpose(1, 0, 2)
        wvc = np.ascontiguousarray(
            pack.reshape(NGRP, GRP, 128, BW)
            .transpose(0, 2, 1, 3)
            .astype(BF16)
        ).reshape(NGRP, 128, GRP * BW)
        in_maps.append({"wv": wvc})
    return in_maps


def kernel(u, d1, d2, v1, v2):
    from concourse.bass_utils import run_bass_kernel_spmd

    u = np.ascontiguousarray(np.asarray(u, np.float32))
    d1 = np.ascontiguousarray(np.asarray(d1, np.float32))
    d2 = np.ascontiguousarray(np.asarray(d2, np.float32))
    v1 = np.ascontiguousarray(np.asarray(v1, np.float32))
    v2 = np.ascontiguousarray(np.asarray(v2, np.float32))

    in_maps = _make_in_maps(u, d1, d2, v1, v2)

    if "nc" not in _NC_CACHE:
        _NC_CACHE["nc"] = _build_nc()
    res = run_bass_kernel_spmd(_NC_CACHE["nc"], in_maps, list(range(N_CORES)))

    # o[g, t, bi*512 + r] per core  ->  out[t, b_global, r]
    out = np.concatenate(
        [
            res.results[c]["o"]
            .astype(np.float32)
            .reshape(NGRP, T, GRP, R)
            .transpose(1, 0, 2, 3)
            .reshape(T, BSH, R)
            for c in range(N_CORES)
        ],
        axis=1,
    )
    return np.ascontiguousarray(out)


if __name__ == "__main__":
    rng = np.random.default_rng(0)
    ins = {
        "u": rng.random((T, B, 1), dtype=np.float32),
        "d1": rng.random((T, B, 1), dtype=np.float32),
        "d2": rng.random((T, B, 1), dtype=np.float32),
        "v1": rng.standard_normal((T, B, R), dtype=np.float32),
        "v2": rng.standard_normal((T, B, R), dtype=np.float32),
    }
    out = kernel(**ins)
    print(out.shape, out.dtype)


# revision 4
# speedup vs baseline: 13.4280x; 13.4280x over previous
"""Stack-style neural memory kernel for Trainium2 (8 NeuronCores, SPMD).

Reference semantics: at step t, push (d1,v1),(d2,v2); read up to total
strength u_t from the top of the stack; pop strength u_t.  The read
summary is linear in the pushed values:

    out[t,b,:] = sum_j W[t,j,b] * V[j,b,:]      (j = slot index, 2T slots)

where the weights W depend only on the (T,B,1)-sized strength tensors
(u,d1,d2).  W is computed on host (tiny sequential bookkeeping; it also
needs a global max over the whole batch, which would otherwise force
cross-core communication).  The device does the memory-heavy part: per
batch element a (T x 2T) @ (2T x R) matmul, batch-parallel across 8
cores with no communication.

Traffic per core: V int8 (per-(slot,batch)-row scales folded into W on
host) 2.1MB + W bf16 1MB in, out bf16 2.1MB back = 5.25MB -> ~14.6us at
the ~358GB/s HBM limit.  V is cast int8->bf16 in flight by the SWDGE
(gpsimd) DMA path; the PE then runs plain bf16 matmuls (1 cycle/row,
~6.8us, hidden under DMA).  Quantization error ~1% rms, well under the
2e-2 gate.
"""

import numpy as np
import ml_dtypes

BF16 = ml_dtypes.bfloat16

T, B, R = 128, 128, 512
NSLOTS = 2 * T
N_CORES = 8
BSH = B // N_CORES  # batch shard per core
GRP = 4             # batches per DMA group
NGRP = BSH // GRP
WM = 256            # per-batch lhsT columns: chunk0 + chunk1
VW = 2 * 512        # per-batch v row: both chunks

_NC_CACHE = {}


def _compute_weights(u, d1, d2):
    """W[t, j, b]: read weight of slot j at step t (float32 (T, 2T, B))."""
    uu = u[:, :, 0]
    S = np.zeros((NSLOTS, B), np.float32)
    W = np.empty((T, NSLOTS, B), np.float32)
    for t in range(T):
        S[2 * t] = d1[t, :, 0]
        S[2 * t + 1] = d2[t, :, 0]
        # strength of slots above j (stack top = highest index first)
        c = np.cumsum(S[::-1], axis=0)[::-1]
        cum = c - S
        avail = uu[t][None, :] - cum
        # reference takes a GLOBAL max over the batch for the read scale
        scal = avail.max(axis=1)
        Wt = np.minimum(S, scal[:, None])
        Wt[2 * t + 2:] = 0.0  # slots not yet pushed hold V=0 in the reference
        W[t] = Wt
        # pop u_t: elementwise depletion, same slot order, same cum
        S -= np.minimum(S, np.maximum(0.0, avail))
    return W


def _build_nc(reps=1, loop_n=1):
    import contextlib

    from concourse import bacc, tile, mybir

    DT = mybir.dt.bfloat16
    nc = bacc.Bacc(None)
    # V, int8-quantized per (slot, batch) row: vq[g, k, bi*VW + c]
    # (c<512: chunk0 slot k, c>=512: chunk1 slot 128+k).  4KB contiguous
    # per partition per group.  Cast int8->bf16 in flight by SWDGE.
    vq = nc.declare_dram_parameter("vq", [NGRP, 128, GRP * VW], mybir.dt.int8,
                                   isOutput=False)
    # W' = W * v_scale, bf16 lhsT pack: w[g, k, bi*WM + m]
    # (m<128: chunk0 lhsT col t=m, m>=128: chunk1 lhsT col t=m-128)
    w = nc.declare_dram_parameter("w", [NGRP, 128, GRP * WM], DT, isOutput=False)
    # output t-major per group: o[g, t, bi*512 + r] (4KB runs per partition)
    o = nc.declare_dram_parameter("o", [NGRP, 128, GRP * 512], DT, isOutput=True)

    with tile.TileContext(nc) as tc:
        with (
            tc.tile_pool(name="vp", bufs=3) as vp,
            tc.tile_pool(name="wp", bufs=3) as wp,
            tc.tile_pool(name="op", bufs=3) as op,
            tc.tile_pool(name="ps", bufs=8, space="PSUM") as ps,
        ):
            loop_cm = (
                tc.For_i(0, loop_n, 1) if loop_n > 1 else contextlib.nullcontext()
            )
            with loop_cm:
                for rep in range(reps):
                    for g in range(NGRP):
                        # alternate the two HWDGE rings (SP / Act) by group
                        # parity so W loads and stores stream on both rings
                        ld = nc.sync if g % 2 == 0 else nc.scalar
                        st = nc.scalar if g % 2 == 0 else nc.sync
                        v_t = vp.tile([128, GRP, VW], DT, tag="v")
                        # SWDGE cast-DMA: HBM int8 -> SBUF bf16
                        nc.gpsimd.dma_start(
                            v_t[:], vq[g].rearrange("k (b c) -> k b c", c=VW)
                        )
                        w_t = wp.tile([128, GRP, WM], DT, tag="w")
                        ld.dma_start(
                            w_t[:], w[g].rearrange("k (b m) -> k b m", m=WM)
                        )
                        out_t = op.tile([128, GRP, 512], DT, tag="out")
                        for bi in range(GRP):
                            acc = ps.tile([128, 512], mybir.dt.float32)
                            nc.tensor.matmul(
                                acc[:],
                                w_t[:, bi, 0:128],
                                v_t[:, bi, 0:512],
                                start=True,
                                stop=False,
                            )
                            nc.tensor.matmul(
                                acc[:],
                                w_t[:, bi, 128:256],
                                v_t[:, bi, 512:1024],
                                start=False,
                                stop=True,
                            )
                            nc.vector.tensor_copy(out_t[:, bi], acc[:])
                        st.dma_start(o[g], out_t[:])
    nc.compile()
    return nc


def _make_in_maps(u, d1, d2, v1, v2):
    W = _compute_weights(u, d1, d2)  # (T, 2T, B)

    Vfull = np.empty((NSLOTS, B, R), np.float32)
    Vfull[0::2] = v1
    Vfull[1::2] = v2

    # int8 quantization of V per (slot, batch) row; scales fold into W
    vs = np.abs(Vfull).max(axis=2) / 127.0          # (2T, B)
    vs[vs == 0.0] = 1.0
    Vq = np.rint(Vfull / vs[:, :, None]).astype(np.int8)
    Ws = W * vs[None, :, :]                          # (T, 2T, B) * s[j,b]

    in_maps = []
    for c in range(N_CORES):
        gb = slice(c * BSH, (c + 1) * BSH)
        Wc = Ws[:, :, gb]         # (T, 256, BSH)
        Vc = Vq[:, gb, :]         # (256, BSH, R) int8
        wpack = np.empty((BSH, 128, WM), np.float32)
        wpack[:, :, 0:128] = Wc[:, 0:128, :].transpose(2, 1, 0)
        wpack[:, :, 128:256] = Wc[:, 128:256, :].transpose(2, 1, 0)
        wc = np.ascontiguousarray(
            wpack.reshape(NGRP, GRP, 128, WM).transpose(0, 2, 1, 3).astype(BF16)
        ).reshape(NGRP, 128, GRP * WM)
        vpack = np.empty((BSH, 128, VW), np.int8)
        vpack[:, :, 0:512] = Vc[0:128].transpose(1, 0, 2)
        vpack[:, :, 512:1024] = Vc[128:256].transpose(1, 0, 2)
        vc = np.ascontiguousarray(
            vpack.reshape(NGRP, GRP, 128, VW).transpose(0, 2, 1, 3)
        ).reshape(NGRP, 128, GRP * VW)
        in_maps.append({"vq": vc, "w": wc})
    return in_maps


def kernel(u, d1, d2, v1, v2):
    from concourse.bass_utils import run_bass_kernel_spmd

    u = np.ascontiguousarray(np.asarray(u, np.float32))
    d1 = np.ascontiguousarray(np.asarray(d1, np.float32))
    d2 = np.ascontiguousarray(np.asarray(d2, np.float32))
    v1 = np.ascontiguousarray(np.asarray(v1, np.float32))
    v2 = np.ascontiguousarray(np.asarray(v2, np.float32))

    in_maps = _make_in_maps(u, d1, d2, v1, v2)

    if "nc" not in _NC_CACHE:
        _NC_CACHE["nc"] = _build_nc()
    res = run_bass_kernel_spmd(_NC_CACHE["nc"], in_maps, list(range(N_CORES)))

    # o[g, t, bi*512 + r] per core  ->  out[t, b_global, r]
    out = np.concatenate(
        [
            res.results[c]["o"]
            .astype(np.float32)
            .reshape(NGRP, T, GRP, R)
            .transpose(1, 0, 2, 3)
            .reshape(T, BSH, R)
            for c in range(N_CORES)
        ],
        axis=1,
    )
    return np.ascontiguousarray(out)


if __name__ == "__main__":
    rng = np.random.default_rng(0)
    ins = {
        "u": rng.random((T, B, 1), dtype=np.float32),
        "d1": rng.random((T, B, 1), dtype=np.float32),
        "d2": rng.random((T, B, 1), dtype=np.float32),
        "v1": rng.standard_normal((T, B, R), dtype=np.float32),
        "v2": rng.standard_normal((T, B, R), dtype=np.float32),
    }
    out = kernel(**ins)
    print(out.shape, out.dtype)


# revision 12
# speedup vs baseline: 22.8820x; 1.7040x over previous
"""Stack-style neural memory kernel for Trainium2 (8 NeuronCores, SPMD).

Reference semantics: at step t, push (d1,v1),(d2,v2); read up to total
strength u_t from the top of the stack; pop strength u_t.  The read
summary is linear in the pushed values:

    out[t,b,:] = sum_j W[t,j,b] * V[j,b,:]      (j = slot index, 2T slots)

where the weights W depend only on the (T,B,1)-sized strength tensors
(u,d1,d2).  W is computed on host (tiny sequential bookkeeping; it also
needs a global max over the whole batch, which would otherwise force
cross-core communication).  The device does the memory-heavy part: per
batch element a (T x 2T) @ (2T x R) matmul, batch-parallel across 8
cores with no communication.

Bottlenecks on trn2 (measured + cost model): all DMAs serialize through
one 16-engine SDMA pool at ~360GB/s of *destination* bytes, and each
dma_start costs ~0.6us of serialized HWDGE descriptor-gen.  So:
  - V ships int8 and lands int8 in SBUF (2.1MB/core; per-(slot,batch)
    row scales folded into W on host); DVE/ACT/GPSIMD upcast to bf16
    through the engine-side SBUF ports, which don't touch the DMA pool.
  - W ships bf16 (1MB/core) with BOTH the V scales and the inverse
    output scale 1/so[t,b] folded in, so the PSUM result is already
    out/so and the PSUM->SBUF copy is a plain saturating-cast copy.
  - out is int8 (1MB/core back); host multiplies by so.  so[t,b] =
    4.6 sigma / 127 with sigma computed exactly from W,V row moments
    (out rows are exactly Gaussian over r), clipping ~3e-6 of elements.
  - Few, large DMAs: flat [128, BSH*...] layouts, halved for pipelining;
    6 dma_starts per pass.
Per-core wire ~4.2MB -> ~11.7us DMA floor; PE ~8us, DVE ~10us, ACT ~9us
overlap it.  Quantization error ~1.3% rms, under the 2e-2 gate.
"""

import numpy as np
import ml_dtypes

BF16 = ml_dtypes.bfloat16

T, B, R = 128, 128, 512
NSLOTS = 2 * T
N_CORES = 8
BSH = B // N_CORES  # batch shard per core
WM = 256            # per-batch lhsT columns: chunk0 + chunk1
VW = 2 * 512        # per-batch v row: both chunks
SIGMA_MULT = 4.6    # int8 out scale = SIGMA_MULT * sigma / 127

_NC_CACHE = {}


def _compute_weights(u, d1, d2):
    """W[t, j, b]: read weight of slot j at step t (float32 (T, 2T, B))."""
    uu = u[:, :, 0]
    S = np.zeros((NSLOTS, B), np.float32)
    W = np.empty((T, NSLOTS, B), np.float32)
    for t in range(T):
        S[2 * t] = d1[t, :, 0]
        S[2 * t + 1] = d2[t, :, 0]
        # strength of slots above j (stack top = highest index first)
        c = np.cumsum(S[::-1], axis=0)[::-1]
        cum = c - S
        avail = uu[t][None, :] - cum
        # reference takes a GLOBAL max over the batch for the read scale
        scal = avail.max(axis=1)
        Wt = np.minimum(S, scal[:, None])
        Wt[2 * t + 2:] = 0.0  # slots not yet pushed hold V=0 in the reference
        W[t] = Wt
        # pop u_t: elementwise depletion, same slot order, same cum
        S -= np.minimum(S, np.maximum(0.0, avail))
    return W


def _build_nc(reps=1, loop_n=1):
    import contextlib

    from concourse import bacc, tile, mybir

    DT = mybir.dt.bfloat16
    H = BSH // 2
    nc = bacc.Bacc(None)
    # flat per-partition layouts; per-batch slices are views
    vq = nc.declare_dram_parameter("vq", [128, BSH * VW], mybir.dt.int8,
                                   isOutput=False)
    w = nc.declare_dram_parameter("w", [128, BSH * WM], DT, isOutput=False)
    o = nc.declare_dram_parameter("o", [128, BSH * 512], mybir.dt.int8,
                                  isOutput=True)

    # static engine assignment, balanced by per-tile cost (DVE cast 594ns,
    # ACT cast 1038, Pool cast 1517; pair-copy ACT 1038, DVE 1191):
    # DVE ~8.3us, ACT ~7.3us, Pool ~7.6us -- all under the ~11.7us DMA floor
    CAST = ["v", "p", "v", "v", "v", "p", "v", "v",
            "p", "v", "v", "p", "v", "a", "v", "p"]
    COPY = ["a", "a", "v", "a", "a", "v", "a", "a"]  # per batch PAIR

    with tile.TileContext(nc) as tc:
        with (
            tc.tile_pool(name="v8p", bufs=3) as v8p,
            tc.tile_pool(name="vp", bufs=2) as vp,
            tc.tile_pool(name="wp", bufs=3) as wp,
            tc.tile_pool(name="op", bufs=2) as op,
            tc.tile_pool(name="ps", bufs=4, space="PSUM") as ps,
        ):
            loop_cm = (
                tc.For_i(0, loop_n, 1) if loop_n > 1 else contextlib.nullcontext()
            )
            with loop_cm:
                for rep in range(reps):
                    v8_t = v8p.tile([128, BSH, VW], mybir.dt.int8, tag="v8")
                    v_t = vp.tile([128, BSH, VW], DT, tag="v")
                    w_t = wp.tile([128, BSH, WM], DT, tag="w")
                    out_t = op.tile([128, BSH, 512], mybir.dt.int8, tag="out")
                    vqv = vq.rearrange("k (b c) -> k b c", c=VW)
                    wv = w.rearrange("k (b m) -> k b m", m=WM)
                    ov = o.rearrange("k (b r) -> k b r", r=512)

                    def cast(bi):
                        e = CAST[bi]
                        if e == "v":
                            nc.vector.tensor_copy(v_t[:, bi], v8_t[:, bi])
                        elif e == "a":
                            nc.scalar.copy(v_t[:, bi], v8_t[:, bi])
                        else:
                            nc.gpsimd.tensor_copy(v_t[:, bi], v8_t[:, bi])

                    def compute_pair(pi):
                        # two batches share one 2-bank PSUM tile; one fused
                        # saturating-cast copy drains both
                        acc = ps.tile([128, 1024], mybir.dt.float32)
                        for half in range(2):
                            bi = 2 * pi + half
                            s = slice(512 * half, 512 * half + 512)
                            nc.tensor.matmul(acc[:, s], w_t[:, bi, 0:128],
                                             v_t[:, bi, 0:512],
                                             start=True, stop=False)
                            nc.tensor.matmul(acc[:, s], w_t[:, bi, 128:256],
                                             v_t[:, bi, 512:1024],
                                             start=False, stop=True)
                        dst = out_t[:, 2 * pi : 2 * pi + 2]
                        if COPY[pi] == "a":
                            nc.scalar.copy(dst, acc[:])
                        else:
                            nc.vector.tensor_copy(dst, acc[:])

                    # V8 quarters stream in; casts/MMs chase each quarter.
                    # W halves land between quarters (needed only by MMs).
                    Q = BSH // 4
                    nc.sync.dma_start(v8_t[:, 0:Q], vqv[:, 0:Q])
                    nc.scalar.dma_start(w_t[:, 0:H], wv[:, 0:H])
                    nc.sync.dma_start(v8_t[:, Q:2 * Q], vqv[:, Q:2 * Q])
                    for bi in range(0, 4):
                        cast(bi)
                    nc.sync.dma_start(v8_t[:, 2 * Q:3 * Q], vqv[:, 2 * Q:3 * Q])
                    for bi in range(4, 8):
                        cast(bi)
                    for pi in range(0, 2):
                        compute_pair(pi)
                    nc.sync.dma_start(v8_t[:, 3 * Q:], vqv[:, 3 * Q:])
                    nc.scalar.dma_start(w_t[:, H:], wv[:, H:])
                    for bi in range(8, 12):
                        cast(bi)
                    for pi in range(2, 4):
                        compute_pair(pi)
                    for bi in range(12, 16):
                        cast(bi)
                    nc.sync.dma_start(ov[:, 0:H], out_t[:, 0:H])
                    for pi in range(4, 8):
                        compute_pair(pi)
                    nc.scalar.dma_start(ov[:, H:], out_t[:, H:])
    nc.compile()
    return nc


def _make_in_maps(u, d1, d2, v1, v2):
    W = _compute_weights(u, d1, d2)  # (T, 2T, B)

    Vfull = np.empty((NSLOTS, B, R), np.float32)
    Vfull[0::2] = v1
    Vfull[1::2] = v2

    # int8 quantization of V per (slot, batch) row; scales fold into W
    vs = np.abs(Vfull).max(axis=2) / 127.0          # (2T, B)
    vs[vs == 0.0] = 1.0
    Vq8 = np.rint(Vfull / vs[:, :, None]).astype(np.int8)
    Ws = (W * vs[None, :, :]).astype(BF16).astype(np.float32)

    # exact second moment of each output row -> int8 out scale, folded
    # into W so the device matmul directly produces out/so
    p2 = (Vq8.astype(np.float32) ** 2).mean(axis=2)  # (2T, B)
    sig = np.sqrt(np.einsum("tjb,jb->tb", Ws * Ws, p2))  # (T, B)
    so = SIGMA_MULT * sig / 127.0
    so[so == 0.0] = 1.0
    Wso = Ws / so[:, None, :]                        # (T, 2T, B)

    in_maps = []
    for c in range(N_CORES):
        gb = slice(c * BSH, (c + 1) * BSH)
        Wc = Wso[:, :, gb]        # (T, 256, BSH)
        Vc = Vq8[:, gb, :]        # (256, BSH, R) int8
        wpack = np.empty((128, BSH, WM), np.float32)
        wpack[:, :, 0:128] = Wc[:, 0:128, :].transpose(1, 2, 0)
        wpack[:, :, 128:256] = Wc[:, 128:256, :].transpose(1, 2, 0)
        vpack = np.empty((128, BSH, VW), np.int8)
        vpack[:, :, 0:512] = Vc[0:128].transpose(0, 1, 2)
        vpack[:, :, 512:1024] = Vc[128:256].transpose(0, 1, 2)
        in_maps.append({
            "vq": np.ascontiguousarray(vpack).reshape(128, BSH * VW),
            "w": np.ascontiguousarray(wpack.astype(BF16)).reshape(128, BSH * WM),
        })
    return in_maps, so


def kernel(u, d1, d2, v1, v2):
    from concourse.bass_utils import run_bass_kernel_spmd

    u = np.ascontiguousarray(np.asarray(u, np.float32))
    d1 = np.ascontiguousarray(np.asarray(d1, np.float32))
    d2 = np.ascontiguousarray(np.asarray(d2, np.float32))
    v1 = np.ascontiguousarray(np.asarray(v1, np.float32))
    v2 = np.ascontiguousarray(np.asarray(v2, np.float32))

    in_maps, so = _make_in_maps(u, d1, d2, v1, v2)

    if "nc" not in _NC_CACHE:
        _NC_CACHE["nc"] = _build_nc()
    res = run_bass_kernel_spmd(_NC_CACHE["nc"], in_maps, list(range(N_CORES)))

    # o[t, b*512 + r] int8 -> dequant -> out[t, b_global, r]
    outs = []
    for c in range(N_CORES):
        gb = slice(c * BSH, (c + 1) * BSH)
        oc = res.results[c]["o"].astype(np.float32).reshape(T, BSH, R)
        outs.append(oc * so[:, gb, None])
    return np.ascontiguousarray(np.concatenate(outs, axis=1))


if __name__ == "__main__":
    rng = np.random.default_rng(0)
    ins = {
        "u": rng.random((T, B, 1), dtype=np.float32),
        "d1": rng.random((T, B, 1), dtype=np.float32),
        "d2": rng.random((T, B, 1), dtype=np.float32),
        "v1": rng.standard_normal((T, B, R), dtype=np.float32),
        "v2": rng.standard_normal((T, B, R), dtype=np.float32),
    }
    out = kernel(**ins)
    print(out.shape, out.dtype)
